# revision 37
# baseline (speedup 1.0000x reference)
"""DeeperGCN forward on 8 TRN2 NeuronCores (Bass/Tile).

Sharding: nodes by range across 8 cores (6250 each); per layer the gather
table (bf16) is replicated into each core's DRAM via AllGather, split in
two halves A (shard rows 0..3071) / B (rows 3072..6249) so AG-A overlaps
the producing layer's tail and AG-B overlaps the consuming layer's head
(A-half gathers are prefetched several supergroups deep and don't wait).
Half-table row indices fit int16 directly (<=25424), so no parity trick.
Edges are partitioned by dst block (128 dsts), split by src half, padded
to 128-edge chunks with counts equalized across cores so one SPMD program
serves all 8.

Edge phase per group of <=8 chunks [128 edges x 128 ch each]:
  xg  = dma_gather(tableA/B, idx)            # bf16, 256B/edge, SWDGE q rotation
  z   = xg + eaW_stream                      # DVE TT (2x mode)
  r   = max(z, 0)                            # DVE tensor_scalar (4x mode)
  w   = exp(t*r)                             # ACT (natural_log_exp set)
  wm  = w*r                                  # DVE TT
  acc_blk += ind_j.T @ [w_j|wm_j]            # PE; ind one-hot host-precomputed
Block finalize: agg = acc_wm * recip(max(acc_w,1e-20)); o = agg + h_blk.
Node phase per block: transpose, W1 matmul, LayerNorm (rsqrt via
exp(-0.5*ln(var+eps)) so the exp/ln/relu/copy set stays resident — zero
ACT table reloads), W2 matmul, residual; next layer's bf16 table shard
relu(LN(x)) is written directly (it doubles as the residual h).
Final layer: relu(LN0(x)) -> head (graph-feature term folded into a
precomputed per-node bias). Output assembled on host.
"""
import sys
import numpy as np

if "/opt/trn_rl_repo" not in sys.path:
    sys.path.insert(0, "/opt/trn_rl_repo")

import ml_dtypes

N = 50000
NC = 8
NPC = N // NC            # 6250
NBLK = 49                # ceil(NPC/128)
NPAD = NBLK * 128        # 6272
NFULL = (NPC // 128) * 128   # 6144 (full blocks)
NTAIL = NPC - NFULL          # 106
NA_BLK = 28              # blocks 0..27 -> table half A (asymmetric: A is the
                         # mid-layer-overlapped AllGather, keep it big; B is
                         # the transition-blocking one, keep it small. NC*NA
                         # must stay < 32768 for int16 gather indices.)
NA = NA_BLK * 128        # 3584 rows per shard in A
NB = NPC - NA            # 2666 rows per shard in B
TA_ROWS = NC * NA        # 28672
TB_ROWS = NC * NB        # 21328
HID = 128
DIN = 64
DE = 16
L = 4
G = 50
GFD = 2
EPS_MSG = 1e-7
LN_EPS = 1e-5
BF16 = ml_dtypes.bfloat16
KGRP = 8                 # chunks per edge-phase group
PREF = 3                 # A-gather prefetch depth (supergroups beyond current)
PREF_B = 2               # B-gather prefetch depth
BURST_A = 8              # A-gathers burst-issued ahead of each collective
BURST_B = 4              # B-gathers burst-issued ahead of ag_A


# --------------------------------------------------------------------------
# host preprocessing
# --------------------------------------------------------------------------

def _prep_edges(src_g, dst_g):
    per = {}
    for c in range(NC):
        lo = c * NPC
        m = (dst_g >= lo) & (dst_g < lo + NPC)
        eid = np.nonzero(m)[0]
        es = src_g[m]
        ed = dst_g[m] - lo
        blk = ed >> 7
        dl = ed & 127
        sc = es // NPC
        sr = es % NPC
        isB = sr >= NA
        tidx = np.where(isB, sc * NB + (sr - NA), sc * NA + sr)
        for b in range(NBLK):
            bm = blk == b
            per[(c, b, 0)] = (eid[bm & ~isB], tidx[bm & ~isB], dl[bm & ~isB])
            per[(c, b, 1)] = (eid[bm & isB], tidx[bm & isB], dl[bm & isB])

    blocks = []
    for b in range(NBLK):
        na = max(max((len(per[(c, b, 0)][0]) + 127) // 128 for c in range(NC)), 1)
        nb = max(max((len(per[(c, b, 1)][0]) + 127) // 128 for c in range(NC)), 1)
        blocks.append((na, nb))

    per_core = []
    for c in range(NC):
        idx_a, idx_b, dstloc, eids = [], [], [], []
        for b in range(NBLK):
            na, nb = blocks[b]
            for par, nch in ((0, na), (1, nb)):
                beid, bidx, bdl = per[(c, b, par)]
                tot = nch * 128
                npad_ = tot - len(bidx)
                idx = np.concatenate([bidx, np.zeros(npad_, np.int64)])
                dl = np.concatenate([bdl, np.full(npad_, -1, np.int64)])
                ei = np.concatenate([beid, np.full(npad_, -1, np.int64)])
                (idx_a if par == 0 else idx_b).append(idx)
                for k in range(nch):
                    dstloc.append(dl[k * 128:(k + 1) * 128])
                    eids.append(ei[k * 128:(k + 1) * 128])

        def wrap(ix):
            a = np.empty((128, len(ix) // 16), np.int16)
            t = ix.reshape(-1, 16).T.astype(np.int16)
            for rep in range(8):
                a[rep * 16:(rep + 1) * 16, :] = t
            return a

        per_core.append(dict(
            idx_a=wrap(np.concatenate(idx_a)),
            idx_b=wrap(np.concatenate(idx_b)),
            dstloc=np.stack(dstloc, axis=1).astype(np.float32),
            eids=eids,
        ))
    return blocks, per_core


def _prep(inputs):
    ii = {k: np.asarray(v) for k, v in inputs.items()}
    src_g = ii['edge_index'][0].astype(np.int64)
    dst_g = ii['edge_index'][1].astype(np.int64)
    blocks, per_core = _prep_edges(src_g, dst_g)

    # eaW = edge_attr @ edge_W + edge_b precomputed on host, streamed bf16
    eaW_full = (ii['edge_attr'].astype(np.float32) @ ii['edge_W'].astype(np.float32)
                + ii['edge_b'].astype(np.float32))           # [E, HID]

    # group boundaries exactly as the kernel's edge loop walks them
    groups = []
    ck = 0
    for b in range(NBLK):
        na, nb = blocks[b]
        for nch in (na, nb):
            g0 = 0
            while g0 < nch:
                k = min(KGRP, nch - g0)
                groups.append((ck + g0, k))
                g0 += k
            ck += nch

    for c in range(NC):
        eids = np.stack(per_core[c]['eids'])                 # [totch, 128]
        vals = eaW_full[np.maximum(eids, 0)]                 # [totch, 128, HID]
        vals[eids < 0] = 0.0
        eaW = vals.transpose(1, 0, 2).reshape(128, -1).astype(BF16)
        dl = per_core[c]['dstloc']                           # [128, totch]
        ind = (dl[:, :, None] ==
               np.arange(128, dtype=np.float32)[None, None, :])
        ind = ind.astype(BF16).reshape(128, -1)
        # interleave per edge group: [eaW k*128 | ind k*128] so one DMA
        # feeds both, with each slice contiguous (keeps DVE 2x mode)
        totch = eaW.shape[1] // 128
        eai = np.empty((128, totch * 256), BF16)
        for ck0, k in groups:
            o = 2 * ck0 * 128
            eai[:, o:o + k * 128] = eaW[:, ck0 * 128:(ck0 + k) * 128]
            eai[:, o + k * 128:o + 2 * k * 128] = ind[:, ck0 * 128:(ck0 + k) * 128]
        per_core[c]['eai'] = eai
        del per_core[c]['eids']
        del per_core[c]['dstloc']

    for c in range(NC):
        sh = ii['x'][c * NPC:(c + 1) * NPC].astype(np.float32)
        xt = np.zeros((DIN, NPAD), np.float32)
        xt[:, :NPC] = sh.T
        per_core[c]['xinT'] = xt

    gf = ii['graph_features'].astype(np.float32)
    npg = N // G
    t = np.repeat(gf.T[:, :, None], npg, axis=2)
    t = t.reshape(G, GFD, npg)
    t = np.transpose(t, (1, 0, 2)).reshape(GFD, G * npg)
    gf_n = t.T
    w0b = ii['head_W0'][HID:HID + GFD].astype(np.float32)
    gfb_full = gf_n @ w0b + ii['head_b0'].astype(np.float32)
    for c in range(NC):
        sh = np.zeros((NPAD, HID), np.float32)
        sh[:NPC] = gfb_full[c * NPC:(c + 1) * NPC]
        per_core[c]['gfb'] = np.ascontiguousarray(
            np.concatenate([sh[b * 128:(b + 1) * 128] for b in range(NBLK)], axis=1))

    W2r = np.concatenate(
        [np.concatenate([ii['W2s'][i][0:128], ii['W2s'][i][128:256]], axis=1)
         for i in range(L)], axis=1).astype(np.float32)  # [128, L*256]

    W = dict(
        node_W=ii['node_W'].astype(np.float32),
        node_b=ii['node_b'].astype(np.float32),
        I128=np.eye(128, dtype=np.float32),
        W1s=np.ascontiguousarray(
            ii['W1s'].astype(np.float32).transpose(1, 0, 2).reshape(128, L * 256)),
        b1s=ii['b1s'].astype(np.float32),
        g1s=ii['g1s'].astype(np.float32),
        be1s=ii['be1s'].astype(np.float32),
        W2s=W2r,
        b2s=ii['b2s'].astype(np.float32),
        ln_gs=ii['ln_gs'].astype(np.float32),
        ln_bs=ii['ln_bs'].astype(np.float32),
        ts=ii['ts'].astype(np.float32),
        head_W0a=ii['head_W0'][:HID].astype(np.float32),
        head_W1=ii['head_W1'].astype(np.float32),
        head_b1=float(np.asarray(ii['head_b1']).reshape(-1)[0]),
    )
    return blocks, per_core, W


# --------------------------------------------------------------------------
# program builder
# --------------------------------------------------------------------------

def _build(blocks, W, n_layers=L, taps_spec=(), max_sg=None):
    import concourse.bass as bass  # noqa: F401
    import concourse.tile as tile
    from concourse import bacc, mybir
    from contextlib import ExitStack

    f32 = mybir.dt.float32
    bf16 = mybir.dt.bfloat16
    i16 = mybir.dt.int16
    AF = mybir.ActivationFunctionType
    ALU = mybir.AluOpType

    tot_a = sum(na for na, _ in blocks) * 128
    tot_b = sum(nb for _, nb in blocks) * 128
    totch = (tot_a + tot_b) // 128

    trivial = (np.allclose(W['ln_gs'], 1) and np.allclose(W['ln_bs'], 0)
               and np.allclose(W['g1s'], 1) and np.allclose(W['be1s'], 0)
               and np.allclose(W['b1s'], 0) and np.allclose(W['b2s'], 0)
               and np.allclose(W['node_b'], 0))
    assert trivial, "non-trivial affine path not implemented"
    assert all(float(t) > 0 for t in W['ts'])

    # Every activation we emit (Exp, Ln, Relu, Copy/Identity) lives in the
    # natural_log_exp_and_others table set; restrict the insertion pass to it
    # so exactly one ACT_TABLE_LOAD is hoisted to the top (the default pass
    # maps Exp->exp_and_others and Ln->natural_log and thrashes ~2.6us per
    # switch, ~190x per layer).
    if not getattr(bacc, "_one_table_patch", False):
        _orig_gat = bacc.get_activation_tables

        def _one_table(arch):
            tabs = _orig_gat(arch)
            if 'natural_log_exp_and_others' not in tabs:
                return tabs
            ours = {f for f in tabs['natural_log_exp_and_others']
                    if str(f).split('.')[-1] in
                    ('Exp', 'Ln', 'Relu', 'Copy', 'Identity', 'Prelu')}
            out = {}
            for k, v in tabs.items():
                if k == 'natural_log_exp_and_others':
                    out[k] = v
                else:
                    out[k] = {f for f in v if f not in ours}
            return out

        bacc.get_activation_tables = _one_table
        bacc._one_table_patch = True

    nc = bacc.Bacc("TRN2", target_bir_lowering=False, debug=False,
                   num_devices=NC, num_swdge_queues=4)

    d = {}
    d['xinT'] = nc.dram_tensor("xinT", [DIN, NPAD], f32, kind="ExternalInput")
    d['idx_a'] = nc.dram_tensor("idx_a", [128, tot_a // 16], i16, kind="ExternalInput")
    d['idx_b'] = nc.dram_tensor("idx_b", [128, tot_b // 16], i16, kind="ExternalInput")
    d['eai'] = nc.dram_tensor("eai", [128, totch * 256], bf16, kind="ExternalInput")
    d['gfb'] = nc.dram_tensor("gfb", [128, NPAD], f32, kind="ExternalInput")
    d['node_W'] = nc.dram_tensor("node_W", [DIN, HID], f32, kind="ExternalInput")
    d['I128'] = nc.dram_tensor("I128", [128, 128], f32, kind="ExternalInput")
    d['W1s'] = nc.dram_tensor("W1s", [128, L * 256], f32, kind="ExternalInput")
    d['W2s'] = nc.dram_tensor("W2s", [128, L * 256], f32, kind="ExternalInput")
    d['head_W0a'] = nc.dram_tensor("head_W0a", [128, 128], f32, kind="ExternalInput")
    d['head_W1'] = nc.dram_tensor("head_W1", [128, 1], f32, kind="ExternalInput")
    d_out = nc.dram_tensor("out", [128, NBLK], f32, kind="ExternalOutput")
    taps = {}
    for name, shape in taps_spec:
        taps[name] = nc.dram_tensor("tap_" + name, list(shape), f32,
                                    kind="ExternalOutput")

    ts_vals = [float(x) for x in W['ts']]

    with ExitStack() as ctx:
        tc = ctx.enter_context(tile.TileContext(nc))
        const = ctx.enter_context(tc.tile_pool(name="const", bufs=1))
        dramp = ctx.enter_context(tc.tile_pool(name="dramp", bufs=1, space="DRAM"))
        big = ctx.enter_context(tc.tile_pool(name="big", bufs=1))
        xgap = ctx.enter_context(tc.tile_pool(name="xga", bufs=BURST_A + 3))
        xgbp = ctx.enter_context(tc.tile_pool(name="xgb", bufs=BURST_B + 1))
        eap = ctx.enter_context(tc.tile_pool(name="ea", bufs=5))
        accp = ctx.enter_context(tc.tile_pool(name="acc", bufs=5, space="PSUM"))
        npsum = ctx.enter_context(tc.tile_pool(name="npsum", bufs=3, space="PSUM"))
        wk = ctx.enter_context(tc.tile_pool(name="wk", bufs=4))
        wcatp = ctx.enter_context(tc.tile_pool(name="wcat", bufs=4))
        nodep = ctx.enter_context(tc.tile_pool(name="node", bufs=4))

        def cload(name, shape, dt):
            t = const.tile(shape, dt, tag=name)
            nc.sync.dma_start(t[:], d[name].ap())
            return t

        c_nodeW = cload('node_W', [DIN, HID], f32)
        c_I = cload('I128', [128, 128], f32)
        c_W1 = cload('W1s', [128, L * 256], f32)
        c_W2 = cload('W2s', [128, L * 256], f32)
        c_hW0a = cload('head_W0a', [128, 128], f32)
        c_hW1 = cload('head_W1', [128, 1], f32)
        c_idx_a = cload('idx_a', [128, tot_a // 16], i16)
        c_idx_b = cload('idx_b', [128, tot_b // 16], i16)

        c_lneps = const.tile([128, 1], f32, tag="lneps", name="lneps")
        nc.gpsimd.memset(c_lneps[:], LN_EPS)
        c_zb = const.tile([128, KGRP * 128], bf16, tag="zb")
        nc.gpsimd.memset(c_zb[:], 0.0)
        xres = big.tile([128, NPAD], bf16, tag="xres")
        hb16 = big.tile([128, NPAD], bf16, tag="hb16")

        shard_A = dramp.tile([NA, HID], bf16, tag="shardA")
        shard_B = dramp.tile([NB, HID], bf16, tag="shardB")
        tablesA = [dramp.tile([TA_ROWS, HID], bf16, tag=f"tableA{i}",
                              name=f"tableA{i}") for i in range(2)]
        tablesB = [dramp.tile([TB_ROWS, HID], bf16, tag=f"tableB{i}",
                              name=f"tableB{i}") for i in range(2)]

        def ag_A(tableA_tile):
            # blocks 0..23 of hb16 -> shard_A rows, then AllGather half A
            nc.sync.dma_start(
                shard_A[:].rearrange("(b p) c -> p b c", p=128),
                hb16[:, 0:NA].rearrange("p (b c) -> p b c", c=HID))
            nc.gpsimd.collective_compute(
                "AllGather", mybir.AluOpType.bypass,
                ins=[shard_A.opt()], outs=[tableA_tile.opt()],
                replica_groups=[list(range(NC))])

        def ag_B(tableB_tile):
            # blocks 24..47 full + tail block 48 -> shard_B, AllGather half B
            nc.sync.dma_start(
                shard_B[0:NFULL - NA, :].rearrange("(b p) c -> p b c", p=128),
                hb16[:, NA:NFULL].rearrange("p (b c) -> p b c", c=HID))
            nc.sync.dma_start(
                shard_B[NFULL - NA:NB, :],
                hb16[0:NTAIL, (NBLK - 1) * 128:(NBLK - 1) * 128 + 128])
            nc.gpsimd.collective_compute(
                "AllGather", mybir.AluOpType.bypass,
                ins=[shard_B.opt()], outs=[tableB_tile.opt()],
                replica_groups=[list(range(NC))])

        def rsqrt_eps(mv, ttag):
            # 1/sqrt(var+eps) = exp(-0.5*ln(var+eps)); ln+exp share one
            # ACT table set with relu/copy -> no table reloads anywhere.
            lv = nodep.tile([128, 1], f32, tag="lv" + ttag)
            nc.scalar.activation(lv[:], mv[:, 1:2], AF.Ln, bias=c_lneps[:, 0:1])
            rs = nodep.tile([128, 1], f32, tag="rs" + ttag)
            nc.scalar.activation(rs[:], lv[:], AF.Exp, scale=-0.5)
            return rs

        def ln_relu(src_ap, out_ap, ttag):
            st = nodep.tile([128, 6], f32, tag="st" + ttag)
            nc.vector.bn_stats(st[:], src_ap)
            mv = nodep.tile([128, 2], f32, tag="mv" + ttag)
            nc.vector.bn_aggr(mv[:], st[:])
            rs = rsqrt_eps(mv, ttag)
            nmb = nodep.tile([128, 1], f32, tag="nm" + ttag)
            nc.vector.tensor_scalar(nmb[:], mv[:, 0:1], rs[:, 0:1], -1.0,
                                    ALU.mult, ALU.mult)
            nc.scalar.activation(out_ap, src_ap, AF.Relu, bias=nmb[:, 0:1],
                                 scale=rs[:, 0:1])

        # ---------------- supergroup table ----------------
        # (blocks, ca, cb, aoff, boff, choff)
        sg_blocks = [list(range(b, min(b + 2, NBLK))) for b in range(0, NBLK, 2)]
        if max_sg is not None:
            sg_blocks = sg_blocks[:max_sg]
        sgs = []
        aoff = boff = choff = 0
        for sg in sg_blocks:
            ca = sum(blocks[b][0] for b in sg)
            cb = sum(blocks[b][1] for b in sg)
            sgs.append((sg, ca, cb, aoff, boff, choff))
            aoff += ca
            boff += cb
            choff += ca + cb
        nsg = len(sgs)
        SI_AGA = (NA_BLK - 1) // 2   # sg index whose last block is block 23

        def split_gather(xg_tile, dst_base, table_tile, idx_c, off, n, q0):
            h = n // 2
            parts = [(0, h), (h, n)] if h > 0 else [(0, n)]
            for pi, (a, bnd) in enumerate(parts):
                cnt = bnd - a
                if cnt <= 0:
                    continue
                nc.gpsimd.dma_gather(
                    xg_tile[:, dst_base + a:dst_base + bnd, :], table_tile[:],
                    idx_c[:, (off + a) * 8:(off + bnd) * 8],
                    cnt * 128, cnt * 128, HID,
                    single_packet=False, queue_num=(q0 + pi) % 4)

        # gather issue state: tiles keyed (li, si), per-layer issue counters
        xga_tiles = {}
        xgb_tiles = {}
        a_issued = [0] * n_layers
        b_issued = [0] * n_layers

        def ensure_A(li_, upto):
            upto = min(upto, nsg)
            while a_issued[li_] < upto:
                si_ = a_issued[li_]
                sg, ca, cb, ao, bo, co = sgs[si_]
                t = xgap.tile([128, ca, HID], bf16, tag="xga")
                split_gather(t, 0, tablesA[li_ % 2], c_idx_a, ao, ca, 0)
                xga_tiles[(li_, si_)] = t
                a_issued[li_] += 1

        def ensure_B(li_, upto):
            upto = min(upto, nsg)
            while b_issued[li_] < upto:
                si_ = b_issued[li_]
                sg, ca, cb, ao, bo, co = sgs[si_]
                t = xgbp.tile([128, cb, HID], bf16, tag="xgb")
                split_gather(t, 0, tablesB[li_ % 2], c_idx_b, bo, cb, 2)
                xgb_tiles[(li_, si_)] = t
                b_issued[li_] += 1

        # ---------------- encoder + table0 ----------------
        for b in range(NBLK):
            xin_t = eap.tile([DIN, 128], f32, tag="xint", name="xin_t")
            nc.sync.dma_start(xin_t[:], d['xinT'].ap()[:, b * 128:(b + 1) * 128])
            ps = npsum.tile([128, 256], f32, tag="nps")
            nc.tensor.matmul(ps[:, 0:HID], xin_t[:],
                             c_nodeW[:], start=True, stop=True)
            nc.vector.tensor_copy(hb16[:, b * 128:(b + 1) * 128], ps[:, 0:HID])
            if b == NA_BLK - 1:
                ag_A(tablesA[0])
        ag_B(tablesB[0])
        # layer-0 A gathers: issued after ag_B's trigger; they wait only on
        # AG-A (done during the encoder tail), so their data streams in while
        # AG-B is still in flight and feeds the A-half edge compute.
        ensure_A(0, BURST_A)

        # ---------------- layers ----------------
        for li in range(n_layers):
            t_imm = ts_vals[li]

            for si in range(nsg):
                ensure_A(li, si + PREF + 1)
                ensure_B(li, si + PREF_B + 1)

                sg, ca, cb, aoff, boff, ch_off = sgs[si]
                xga_flat = xga_tiles.pop((li, si))[:].rearrange("p s c -> p (s c)")
                xgb_flat = xgb_tiles.pop((li, si))[:].rearrange("p s c -> p (s c)")

                sa = 0
                sb = 0
                for b in sg:
                    na, nb = blocks[b]
                    nchb = na + nb
                    acc = accp.tile([128, 256], f32, tag="acc")
                    gstarts = ([(g0, False) for g0 in range(0, na, KGRP)]
                               + [(na + g0, True) for g0 in range(0, nb, KGRP)])
                    for g0, isb in gstarts:
                        within = g0 - na if isb else g0
                        k = min(KGRP, (nb - within) if isb else (na - within))
                        ck0 = ch_off + g0
                        eai_t = eap.tile([128, 2 * KGRP * 128], bf16, tag="eai")
                        nc.sync.dma_start(
                            eai_t[:, 0:2 * k * 128],
                            d['eai'].ap()[:, 2 * ck0 * 128:2 * (ck0 + k) * 128])
                        eaW_t = eai_t[:, 0:k * 128]
                        slot0 = (sb + within) if isb else (sa + within)
                        xg3 = (xgb_flat if isb else xga_flat)[
                            :, slot0 * 128:(slot0 + k) * 128]
                        z_t = wk.tile([128, KGRP * 128], bf16, tag="z")
                        nc.vector.tensor_tensor(z_t[:, 0:k * 128], xg3,
                                                eaW_t, ALU.add)
                        r_t = wk.tile([128, KGRP * 128], bf16, tag="r")
                        nc.vector.tensor_tensor(r_t[:, 0:k * 128],
                                                z_t[:, 0:k * 128],
                                                c_zb[:, 0:k * 128], ALU.max)
                        wcat = wcatp.tile([128, 2, KGRP * 128], bf16, tag="wcat")
                        nc.scalar.activation(wcat[:, 0, 0:k * 128],
                                             r_t[:, 0:k * 128], AF.Exp,
                                             scale=t_imm)
                        nc.vector.tensor_tensor(wcat[:, 1, 0:k * 128],
                                                wcat[:, 0, 0:k * 128],
                                                r_t[:, 0:k * 128], ALU.mult)
                        for j in range(k):
                            nc.tensor.matmul(
                                acc[:], eai_t[:, (k + j) * 128:(k + j + 1) * 128],
                                wcat[:, :, j * 128:(j + 1) * 128],
                                start=(g0 + j == 0), stop=(g0 + j == nchb - 1))

                    # ---- finalize + node phase ----
                    if 'acc0' in taps and li == 0 and b == 0:
                        tap_t = wk.tile([128, 256], f32, tag="tapacc")
                        nc.vector.tensor_copy(tap_t[:], acc[:])
                        nc.sync.dma_start(taps['acc0'].ap(), tap_t[:])
                    s_t = wk.tile([128, 128], f32, tag="s")
                    nc.vector.tensor_scalar_max(s_t[:], acc[:, 0:128], 1e-20)
                    rec = wk.tile([128, 128], f32, tag="rec")
                    nc.vector.reciprocal_approx_fast(rec[:], s_t[:])
                    o_t = nodep.tile([128, 128], f32, tag="o")
                    nc.vector.tensor_tensor(o_t[:], acc[:, 128:256], rec[:],
                                            ALU.mult)
                    nc.vector.tensor_add(o_t[:], o_t[:],
                                         hb16[:, b * 128:(b + 1) * 128])

                    # node phase
                    tps = npsum.tile([128, 256], f32, tag="nps")
                    nc.tensor.transpose(tps[:, 0:128], o_t[:], c_I[:])
                    oT = nodep.tile([128, 128], f32, tag="oT")
                    nc.scalar.copy(oT[:], tps[:, 0:128])
                    ps1 = npsum.tile([128, 256], f32, tag="nps")
                    nc.tensor.matmul(ps1[:], oT[:],
                                     c_W1[:, li * 256:(li + 1) * 256],
                                     start=True, stop=True)
                    st = nodep.tile([128, 6], f32, tag="st1")
                    nc.vector.bn_stats(st[:], ps1[:])
                    mv = nodep.tile([128, 2], f32, tag="mv1")
                    nc.vector.bn_aggr(mv[:], st[:])
                    rs1 = rsqrt_eps(mv, "1")
                    nmb1 = nodep.tile([128, 1], f32, tag="nm1")
                    nc.vector.tensor_scalar(nmb1[:], mv[:, 0:1], rs1[:, 0:1], -1.0,
                                            ALU.mult, ALU.mult)
                    h1 = nodep.tile([128, 256], f32, tag="h1")
                    nc.scalar.activation(h1[:], ps1[:], AF.Relu,
                                         bias=nmb1[:, 0:1], scale=rs1[:, 0:1])
                    h1T = nodep.tile([128, 256], f32, tag="h1T")
                    for hh in range(2):
                        tps2 = npsum.tile([128, 256], f32, tag="nps")
                        nc.tensor.transpose(tps2[:, 0:128],
                                            h1[:, hh * 128:(hh + 1) * 128], c_I[:])
                        nc.scalar.copy(h1T[:, hh * 128:(hh + 1) * 128],
                                       tps2[:, 0:128])
                    ps2 = npsum.tile([128, 256], f32, tag="nps")
                    for hh in range(2):
                        nc.tensor.matmul(
                            ps2[:, 0:128], h1T[:, hh * 128:(hh + 1) * 128],
                            c_W2[:, li * 256 + hh * 128:li * 256 + (hh + 1) * 128],
                            start=(hh == 0), stop=(hh == 1))
                    xblk = xres[:, b * 128:(b + 1) * 128]
                    if li == 0:
                        nc.scalar.copy(xblk, ps2[:, 0:128])
                    else:
                        nc.vector.tensor_add(xblk, xblk, ps2[:, 0:128])

                    if li + 1 < n_layers:
                        ln_relu(xblk, hb16[:, b * 128:(b + 1) * 128], "t")
                    elif n_layers == L:
                        xf = nodep.tile([128, 128], f32, tag="xf")
                        ln_relu(xblk, xf[:], "f")
                        tps3 = npsum.tile([128, 256], f32, tag="nps")
                        nc.tensor.transpose(tps3[:, 0:128], xf[:], c_I[:])
                        xfT = nodep.tile([128, 128], f32, tag="xfT")
                        nc.scalar.copy(xfT[:], tps3[:, 0:128])
                        ph = npsum.tile([128, 256], f32, tag="nps")
                        nc.tensor.matmul(ph[:, 0:128], xfT[:], c_hW0a[:],
                                         start=True, stop=True)
                        gfb_t = eap.tile([128, 128], f32, tag="gfbt", name="gfb_t")
                        nc.sync.dma_start(gfb_t[:],
                                          d['gfb'].ap()[:, b * 128:(b + 1) * 128])
                        hh1 = nodep.tile([128, 128], f32, tag="hh1")
                        nc.vector.tensor_add(hh1[:], ph[:, 0:128], gfb_t[:])
                        nc.vector.tensor_scalar_max(hh1[:], hh1[:], 0.0)
                        tps4 = npsum.tile([128, 256], f32, tag="nps")
                        nc.tensor.transpose(tps4[:, 0:128], hh1[:], c_I[:])
                        hh1T = nodep.tile([128, 128], f32, tag="hh1T")
                        nc.scalar.copy(hh1T[:], tps4[:, 0:128])
                        po = npsum.tile([128, 256], f32, tag="nps")
                        nc.tensor.matmul(po[:, 0:1], hh1T[:], c_hW1[:],
                                         start=True, stop=True)
                        ocol = nodep.tile([128, 1], f32, tag="ocol")
                        nc.vector.tensor_copy(ocol[:], po[:, 0:1])
                        nc.sync.dma_start(d_out.ap()[:, b:b + 1], ocol[:])

                    if li + 1 < n_layers and b == NA_BLK - 1:
                        # burst-prefetch ahead of the collective: gathers
                        # issued after it on gpsimd stall until it completes
                        ensure_A(li, si + 1 + BURST_A)
                        ensure_B(li, si + 1 + BURST_B)
                        ag_A(tablesA[(li + 1) % 2])

                    sa += na
                    sb += nb
                    ch_off += nchb

            if 'x_l%d' % li in taps:
                tap_t = taps['x_l%d' % li]
                nc.sync.dma_start(tap_t.ap(), xres[:])
            if li + 1 < n_layers:
                # next layer's A-half reads tablesA[(li+1)%2], complete since
                # mid-layer; issue its gathers before ag_B so the A-half edge
                # compute rides out the collective.
                ensure_A(li + 1, BURST_A)
                ag_B(tablesB[(li + 1) % 2])

    nc.compile()
    return nc, taps


# --------------------------------------------------------------------------
# entry point
# --------------------------------------------------------------------------

def _in_maps(blocks, per_core, W):
    shared = dict(
        node_W=W['node_W'], I128=W['I128'], W1s=W['W1s'], W2s=W['W2s'],
        head_W0a=W['head_W0a'], head_W1=W['head_W1'])
    return [dict(per_core[c], **shared) for c in range(NC)]


def kernel(**inputs):
    from concourse import bass_utils
    blocks, per_core, W = _prep(inputs)
    nc, _ = _build(blocks, W)
    res = bass_utils.run_bass_kernel_spmd(
        nc, _in_maps(blocks, per_core, W), core_ids=list(range(NC)),
        trace=False)
    out = np.empty((N, 1), np.float32)
    for c in range(NC):
        oc = res.results[c]['out']
        out[c * NPC:(c + 1) * NPC, 0] = oc.T.reshape(-1)[:NPC] + W['head_b1']
    return out


# revision 38
# speedup vs baseline: 1.0230x; 1.0230x over previous
"""DeeperGCN forward on 8 TRN2 NeuronCores (Bass/Tile).

Sharding: nodes by range across 8 cores (6250 each); per layer the gather
table (bf16) is replicated into each core's DRAM via AllGather, split in
two halves A (shard rows 0..3071) / B (rows 3072..6249) so AG-A overlaps
the producing layer's tail and AG-B overlaps the consuming layer's head
(A-half gathers are prefetched several supergroups deep and don't wait).
Half-table row indices fit int16 directly (<=25424), so no parity trick.
Edges are partitioned by dst block (128 dsts), split by src half, padded
to 128-edge chunks with counts equalized across cores so one SPMD program
serves all 8.

Edge phase per group of <=8 chunks [128 edges x 128 ch each]:
  xg  = dma_gather(tableA/B, idx)            # bf16, 256B/edge, SWDGE q rotation
  z   = xg + eaW_stream                      # DVE TT (2x mode)
  r   = max(z, 0)                            # DVE tensor_scalar (4x mode)
  w   = exp(t*r)                             # ACT (natural_log_exp set)
  wm  = w*r                                  # DVE TT
  acc_blk += ind_j.T @ [w_j|wm_j]            # PE; ind one-hot host-precomputed
Block finalize: agg = acc_wm * recip(max(acc_w,1e-20)); o = agg + h_blk.
Node phase per block: transpose, W1 matmul, LayerNorm (rsqrt via
exp(-0.5*ln(var+eps)) so the exp/ln/relu/copy set stays resident — zero
ACT table reloads), W2 matmul, residual; next layer's bf16 table shard
relu(LN(x)) is written directly (it doubles as the residual h).
Final layer: relu(LN0(x)) -> head (graph-feature term folded into a
precomputed per-node bias). Output assembled on host.
"""
import sys
import numpy as np

if "/opt/trn_rl_repo" not in sys.path:
    sys.path.insert(0, "/opt/trn_rl_repo")

import ml_dtypes

N = 50000
NC = 8
NPC = N // NC            # 6250
NBLK = 49                # ceil(NPC/128)
NPAD = NBLK * 128        # 6272
NFULL = (NPC // 128) * 128   # 6144 (full blocks)
NTAIL = NPC - NFULL          # 106
NA_BLK = 28              # blocks 0..27 -> table half A (asymmetric: A is the
                         # mid-layer-overlapped AllGather, keep it big; B is
                         # the transition-blocking one, keep it small. NC*NA
                         # must stay < 32768 for int16 gather indices.)
NA = NA_BLK * 128        # 3584 rows per shard in A
NB = NPC - NA            # 2666 rows per shard in B
TA_ROWS = NC * NA        # 28672
TB_ROWS = NC * NB        # 21328
HID = 128
DIN = 64
DE = 16
L = 4
G = 50
GFD = 2
EPS_MSG = 1e-7
LN_EPS = 1e-5
BF16 = ml_dtypes.bfloat16
KGRP = 8                 # chunks per edge-phase group
PREF = 3                 # A-gather prefetch depth (supergroups beyond current)
PREF_B = 2               # B-gather prefetch depth
BURST_A = 8              # A-gathers burst-issued ahead of each collective
BURST_B = 4              # B-gathers burst-issued ahead of ag_A


# --------------------------------------------------------------------------
# host preprocessing
# --------------------------------------------------------------------------

def _prep_edges(src_g, dst_g):
    per = {}
    for c in range(NC):
        lo = c * NPC
        m = (dst_g >= lo) & (dst_g < lo + NPC)
        eid = np.nonzero(m)[0]
        es = src_g[m]
        ed = dst_g[m] - lo
        blk = ed >> 7
        dl = ed & 127
        sc = es // NPC
        sr = es % NPC
        isB = sr >= NA
        tidx = np.where(isB, sc * NB + (sr - NA), sc * NA + sr)
        for b in range(NBLK):
            bm = blk == b
            per[(c, b, 0)] = (eid[bm & ~isB], tidx[bm & ~isB], dl[bm & ~isB])
            per[(c, b, 1)] = (eid[bm & isB], tidx[bm & isB], dl[bm & isB])

    blocks = []
    for b in range(NBLK):
        na = max(max((len(per[(c, b, 0)][0]) + 127) // 128 for c in range(NC)), 1)
        nb = max(max((len(per[(c, b, 1)][0]) + 127) // 128 for c in range(NC)), 1)
        blocks.append((na, nb))

    per_core = []
    for c in range(NC):
        idx_a, idx_b, dstloc, eids = [], [], [], []
        for b in range(NBLK):
            na, nb = blocks[b]
            for par, nch in ((0, na), (1, nb)):
                beid, bidx, bdl = per[(c, b, par)]
                tot = nch * 128
                npad_ = tot - len(bidx)
                idx = np.concatenate([bidx, np.zeros(npad_, np.int64)])
                dl = np.concatenate([bdl, np.full(npad_, -1, np.int64)])
                ei = np.concatenate([beid, np.full(npad_, -1, np.int64)])
                (idx_a if par == 0 else idx_b).append(idx)
                for k in range(nch):
                    dstloc.append(dl[k * 128:(k + 1) * 128])
                    eids.append(ei[k * 128:(k + 1) * 128])

        def wrap(ix):
            a = np.empty((128, len(ix) // 16), np.int16)
            t = ix.reshape(-1, 16).T.astype(np.int16)
            for rep in range(8):
                a[rep * 16:(rep + 1) * 16, :] = t
            return a

        per_core.append(dict(
            idx_a=wrap(np.concatenate(idx_a)),
            idx_b=wrap(np.concatenate(idx_b)),
            dstloc=np.stack(dstloc, axis=1).astype(np.float32),
            eids=eids,
        ))
    return blocks, per_core


def _prep(inputs):
    ii = {k: np.asarray(v) for k, v in inputs.items()}
    src_g = ii['edge_index'][0].astype(np.int64)
    dst_g = ii['edge_index'][1].astype(np.int64)
    blocks, per_core = _prep_edges(src_g, dst_g)

    # eaW = edge_attr @ edge_W + edge_b precomputed on host, streamed bf16
    eaW_full = (ii['edge_attr'].astype(np.float32) @ ii['edge_W'].astype(np.float32)
                + ii['edge_b'].astype(np.float32))           # [E, HID]

    for c in range(NC):
        eids = np.stack(per_core[c]['eids'])                 # [totch, 128]
        vals = eaW_full[np.maximum(eids, 0)]                 # [totch, 128, HID]
        vals[eids < 0] = 0.0
        per_core[c]['eaW'] = np.ascontiguousarray(
            vals.transpose(1, 0, 2).reshape(128, -1)).astype(BF16)
        dl = per_core[c]['dstloc']                           # [128, totch]
        ind = (dl[:, :, None] ==
               np.arange(128, dtype=np.float32)[None, None, :])
        per_core[c]['ind'] = np.ascontiguousarray(
            ind.astype(BF16).reshape(128, -1))
        del per_core[c]['eids']
        del per_core[c]['dstloc']

    for c in range(NC):
        sh = ii['x'][c * NPC:(c + 1) * NPC].astype(np.float32)
        xt = np.zeros((DIN, NPAD), np.float32)
        xt[:, :NPC] = sh.T
        per_core[c]['xinT'] = xt

    gf = ii['graph_features'].astype(np.float32)
    npg = N // G
    t = np.repeat(gf.T[:, :, None], npg, axis=2)
    t = t.reshape(G, GFD, npg)
    t = np.transpose(t, (1, 0, 2)).reshape(GFD, G * npg)
    gf_n = t.T
    w0b = ii['head_W0'][HID:HID + GFD].astype(np.float32)
    gfb_full = gf_n @ w0b + ii['head_b0'].astype(np.float32)
    for c in range(NC):
        sh = np.zeros((NPAD, HID), np.float32)
        sh[:NPC] = gfb_full[c * NPC:(c + 1) * NPC]
        per_core[c]['gfb'] = np.ascontiguousarray(
            np.concatenate([sh[b * 128:(b + 1) * 128] for b in range(NBLK)], axis=1))

    W2r = np.concatenate(
        [np.concatenate([ii['W2s'][i][0:128], ii['W2s'][i][128:256]], axis=1)
         for i in range(L)], axis=1).astype(np.float32)  # [128, L*256]

    W = dict(
        node_W=ii['node_W'].astype(np.float32),
        node_b=ii['node_b'].astype(np.float32),
        I128=np.eye(128, dtype=np.float32),
        W1s=np.ascontiguousarray(
            ii['W1s'].astype(np.float32).transpose(1, 0, 2).reshape(128, L * 256)),
        b1s=ii['b1s'].astype(np.float32),
        g1s=ii['g1s'].astype(np.float32),
        be1s=ii['be1s'].astype(np.float32),
        W2s=W2r,
        b2s=ii['b2s'].astype(np.float32),
        ln_gs=ii['ln_gs'].astype(np.float32),
        ln_bs=ii['ln_bs'].astype(np.float32),
        ts=ii['ts'].astype(np.float32),
        head_W0a=ii['head_W0'][:HID].astype(np.float32),
        head_W1=ii['head_W1'].astype(np.float32),
        head_b1=float(np.asarray(ii['head_b1']).reshape(-1)[0]),
    )
    return blocks, per_core, W


# --------------------------------------------------------------------------
# program builder
# --------------------------------------------------------------------------

def _build(blocks, W, n_layers=L, taps_spec=(), max_sg=None):
    import concourse.bass as bass  # noqa: F401
    import concourse.tile as tile
    from concourse import bacc, mybir
    from contextlib import ExitStack

    f32 = mybir.dt.float32
    bf16 = mybir.dt.bfloat16
    i16 = mybir.dt.int16
    AF = mybir.ActivationFunctionType
    ALU = mybir.AluOpType

    tot_a = sum(na for na, _ in blocks) * 128
    tot_b = sum(nb for _, nb in blocks) * 128
    totch = (tot_a + tot_b) // 128

    trivial = (np.allclose(W['ln_gs'], 1) and np.allclose(W['ln_bs'], 0)
               and np.allclose(W['g1s'], 1) and np.allclose(W['be1s'], 0)
               and np.allclose(W['b1s'], 0) and np.allclose(W['b2s'], 0)
               and np.allclose(W['node_b'], 0))
    assert trivial, "non-trivial affine path not implemented"
    assert all(float(t) > 0 for t in W['ts'])

    # Every activation we emit (Exp, Ln, Relu, Copy/Identity) lives in the
    # natural_log_exp_and_others table set; restrict the insertion pass to it
    # so exactly one ACT_TABLE_LOAD is hoisted to the top (the default pass
    # maps Exp->exp_and_others and Ln->natural_log and thrashes ~2.6us per
    # switch, ~190x per layer).
    if not getattr(bacc, "_one_table_patch", False):
        _orig_gat = bacc.get_activation_tables

        def _one_table(arch):
            tabs = _orig_gat(arch)
            if 'natural_log_exp_and_others' not in tabs:
                return tabs
            ours = {f for f in tabs['natural_log_exp_and_others']
                    if str(f).split('.')[-1] in
                    ('Exp', 'Ln', 'Relu', 'Copy', 'Identity', 'Prelu')}
            out = {}
            for k, v in tabs.items():
                if k == 'natural_log_exp_and_others':
                    out[k] = v
                else:
                    out[k] = {f for f in v if f not in ours}
            return out

        bacc.get_activation_tables = _one_table
        bacc._one_table_patch = True

    nc = bacc.Bacc("TRN2", target_bir_lowering=False, debug=False,
                   num_devices=NC, num_swdge_queues=4)

    d = {}
    d['xinT'] = nc.dram_tensor("xinT", [DIN, NPAD], f32, kind="ExternalInput")
    d['idx_a'] = nc.dram_tensor("idx_a", [128, tot_a // 16], i16, kind="ExternalInput")
    d['idx_b'] = nc.dram_tensor("idx_b", [128, tot_b // 16], i16, kind="ExternalInput")
    d['eaW'] = nc.dram_tensor("eaW", [128, totch * 128], bf16, kind="ExternalInput")
    d['ind'] = nc.dram_tensor("ind", [128, totch * 128], bf16, kind="ExternalInput")
    d['gfb'] = nc.dram_tensor("gfb", [128, NPAD], f32, kind="ExternalInput")
    d['node_W'] = nc.dram_tensor("node_W", [DIN, HID], f32, kind="ExternalInput")
    d['I128'] = nc.dram_tensor("I128", [128, 128], f32, kind="ExternalInput")
    d['W1s'] = nc.dram_tensor("W1s", [128, L * 256], f32, kind="ExternalInput")
    d['W2s'] = nc.dram_tensor("W2s", [128, L * 256], f32, kind="ExternalInput")
    d['head_W0a'] = nc.dram_tensor("head_W0a", [128, 128], f32, kind="ExternalInput")
    d['head_W1'] = nc.dram_tensor("head_W1", [128, 1], f32, kind="ExternalInput")
    d_out = nc.dram_tensor("out", [128, NBLK], f32, kind="ExternalOutput")
    taps = {}
    for name, shape in taps_spec:
        taps[name] = nc.dram_tensor("tap_" + name, list(shape), f32,
                                    kind="ExternalOutput")

    ts_vals = [float(x) for x in W['ts']]

    with ExitStack() as ctx:
        tc = ctx.enter_context(tile.TileContext(nc))
        const = ctx.enter_context(tc.tile_pool(name="const", bufs=1))
        dramp = ctx.enter_context(tc.tile_pool(name="dramp", bufs=1, space="DRAM"))
        big = ctx.enter_context(tc.tile_pool(name="big", bufs=1))
        xgap = ctx.enter_context(tc.tile_pool(name="xga", bufs=BURST_A + 3))
        xgbp = ctx.enter_context(tc.tile_pool(name="xgb", bufs=BURST_B + 1))
        eap = ctx.enter_context(tc.tile_pool(name="ea", bufs=5))
        indp = ctx.enter_context(tc.tile_pool(name="ind", bufs=5))
        accp = ctx.enter_context(tc.tile_pool(name="acc", bufs=5, space="PSUM"))
        npsum = ctx.enter_context(tc.tile_pool(name="npsum", bufs=3, space="PSUM"))
        wk = ctx.enter_context(tc.tile_pool(name="wk", bufs=4))
        wcatp = ctx.enter_context(tc.tile_pool(name="wcat", bufs=4))
        nodep = ctx.enter_context(tc.tile_pool(name="node", bufs=4))

        def cload(name, shape, dt):
            t = const.tile(shape, dt, tag=name)
            nc.sync.dma_start(t[:], d[name].ap())
            return t

        c_nodeW = cload('node_W', [DIN, HID], f32)
        c_I = cload('I128', [128, 128], f32)
        c_W1 = cload('W1s', [128, L * 256], f32)
        c_W2 = cload('W2s', [128, L * 256], f32)
        c_hW0a = cload('head_W0a', [128, 128], f32)
        c_hW1 = cload('head_W1', [128, 1], f32)
        c_idx_a = cload('idx_a', [128, tot_a // 16], i16)
        c_idx_b = cload('idx_b', [128, tot_b // 16], i16)

        c_lneps = const.tile([128, 1], f32, tag="lneps", name="lneps")
        nc.gpsimd.memset(c_lneps[:], LN_EPS)
        c_zb = const.tile([128, KGRP * 128], bf16, tag="zb")
        nc.gpsimd.memset(c_zb[:], 0.0)
        xres = big.tile([128, NPAD], bf16, tag="xres")
        hb16 = big.tile([128, NPAD], bf16, tag="hb16")

        shard_A = dramp.tile([NA, HID], bf16, tag="shardA")
        shard_B = dramp.tile([NB, HID], bf16, tag="shardB")
        tablesA = [dramp.tile([TA_ROWS, HID], bf16, tag=f"tableA{i}",
                              name=f"tableA{i}") for i in range(2)]
        tablesB = [dramp.tile([TB_ROWS, HID], bf16, tag=f"tableB{i}",
                              name=f"tableB{i}") for i in range(2)]

        def ag_A(tableA_tile):
            # blocks 0..23 of hb16 -> shard_A rows, then AllGather half A
            nc.sync.dma_start(
                shard_A[:].rearrange("(b p) c -> p b c", p=128),
                hb16[:, 0:NA].rearrange("p (b c) -> p b c", c=HID))
            nc.gpsimd.collective_compute(
                "AllGather", mybir.AluOpType.bypass,
                ins=[shard_A.opt()], outs=[tableA_tile.opt()],
                replica_groups=[list(range(NC))])

        def ag_B(tableB_tile):
            # blocks 24..47 full + tail block 48 -> shard_B, AllGather half B
            nc.sync.dma_start(
                shard_B[0:NFULL - NA, :].rearrange("(b p) c -> p b c", p=128),
                hb16[:, NA:NFULL].rearrange("p (b c) -> p b c", c=HID))
            nc.sync.dma_start(
                shard_B[NFULL - NA:NB, :],
                hb16[0:NTAIL, (NBLK - 1) * 128:(NBLK - 1) * 128 + 128])
            nc.gpsimd.collective_compute(
                "AllGather", mybir.AluOpType.bypass,
                ins=[shard_B.opt()], outs=[tableB_tile.opt()],
                replica_groups=[list(range(NC))])

        def rsqrt_eps(mv, ttag):
            # 1/sqrt(var+eps) = exp(-0.5*ln(var+eps)); ln+exp share one
            # ACT table set with relu/copy -> no table reloads anywhere.
            lv = nodep.tile([128, 1], f32, tag="lv" + ttag)
            nc.scalar.activation(lv[:], mv[:, 1:2], AF.Ln, bias=c_lneps[:, 0:1])
            rs = nodep.tile([128, 1], f32, tag="rs" + ttag)
            nc.scalar.activation(rs[:], lv[:], AF.Exp, scale=-0.5)
            return rs

        def ln_relu(src_ap, out_ap, ttag):
            st = nodep.tile([128, 6], f32, tag="st" + ttag)
            nc.vector.bn_stats(st[:], src_ap)
            mv = nodep.tile([128, 2], f32, tag="mv" + ttag)
            nc.vector.bn_aggr(mv[:], st[:])
            rs = rsqrt_eps(mv, ttag)
            nmb = nodep.tile([128, 1], f32, tag="nm" + ttag)
            nc.vector.tensor_scalar(nmb[:], mv[:, 0:1], rs[:, 0:1], -1.0,
                                    ALU.mult, ALU.mult)
            nc.scalar.activation(out_ap, src_ap, AF.Relu, bias=nmb[:, 0:1],
                                 scale=rs[:, 0:1])

        # ---------------- supergroup table ----------------
        # (blocks, ca, cb, aoff, boff, choff)
        sg_blocks = [list(range(b, min(b + 2, NBLK))) for b in range(0, NBLK, 2)]
        if max_sg is not None:
            sg_blocks = sg_blocks[:max_sg]
        sgs = []
        aoff = boff = choff = 0
        for sg in sg_blocks:
            ca = sum(blocks[b][0] for b in sg)
            cb = sum(blocks[b][1] for b in sg)
            sgs.append((sg, ca, cb, aoff, boff, choff))
            aoff += ca
            boff += cb
            choff += ca + cb
        nsg = len(sgs)
        SI_AGA = (NA_BLK - 1) // 2   # sg index whose last block is block 23

        def split_gather(xg_tile, dst_base, table_tile, idx_c, off, n, q0):
            h = n // 2
            parts = [(0, h), (h, n)] if h > 0 else [(0, n)]
            for pi, (a, bnd) in enumerate(parts):
                cnt = bnd - a
                if cnt <= 0:
                    continue
                nc.gpsimd.dma_gather(
                    xg_tile[:, dst_base + a:dst_base + bnd, :], table_tile[:],
                    idx_c[:, (off + a) * 8:(off + bnd) * 8],
                    cnt * 128, cnt * 128, HID,
                    single_packet=False, queue_num=(q0 + pi) % 4)

        # gather issue state: tiles keyed (li, si), per-layer issue counters
        xga_tiles = {}
        xgb_tiles = {}
        a_issued = [0] * n_layers
        b_issued = [0] * n_layers

        def ensure_A(li_, upto):
            upto = min(upto, nsg)
            while a_issued[li_] < upto:
                si_ = a_issued[li_]
                sg, ca, cb, ao, bo, co = sgs[si_]
                t = xgap.tile([128, ca, HID], bf16, tag="xga")
                split_gather(t, 0, tablesA[li_ % 2], c_idx_a, ao, ca, 0)
                xga_tiles[(li_, si_)] = t
                a_issued[li_] += 1

        def ensure_B(li_, upto):
            upto = min(upto, nsg)
            while b_issued[li_] < upto:
                si_ = b_issued[li_]
                sg, ca, cb, ao, bo, co = sgs[si_]
                t = xgbp.tile([128, cb, HID], bf16, tag="xgb")
                split_gather(t, 0, tablesB[li_ % 2], c_idx_b, bo, cb, 2)
                xgb_tiles[(li_, si_)] = t
                b_issued[li_] += 1

        # ---------------- encoder + table0 ----------------
        for b in range(NBLK):
            xin_t = eap.tile([DIN, 128], f32, tag="xint", name="xin_t")
            nc.sync.dma_start(xin_t[:], d['xinT'].ap()[:, b * 128:(b + 1) * 128])
            ps = npsum.tile([128, 256], f32, tag="nps")
            nc.tensor.matmul(ps[:, 0:HID], xin_t[:],
                             c_nodeW[:], start=True, stop=True)
            nc.vector.tensor_copy(hb16[:, b * 128:(b + 1) * 128], ps[:, 0:HID])
            if b == NA_BLK - 1:
                ag_A(tablesA[0])
        ag_B(tablesB[0])
        # layer-0 A gathers: issued after ag_B's trigger; they wait only on
        # AG-A (done during the encoder tail), so their data streams in while
        # AG-B is still in flight and feeds the A-half edge compute.
        ensure_A(0, BURST_A)

        # ---------------- layers ----------------
        for li in range(n_layers):
            t_imm = ts_vals[li]

            for si in range(nsg):
                ensure_A(li, si + PREF + 1)
                ensure_B(li, si + PREF_B + 1)

                sg, ca, cb, aoff, boff, ch_off = sgs[si]
                xga_flat = xga_tiles.pop((li, si))[:].rearrange("p s c -> p (s c)")
                xgb_flat = xgb_tiles.pop((li, si))[:].rearrange("p s c -> p (s c)")

                sa = 0
                sb = 0
                for b in sg:
                    na, nb = blocks[b]
                    nchb = na + nb
                    acc = accp.tile([128, 256], f32, tag="acc")
                    gstarts = ([(g0, False) for g0 in range(0, na, KGRP)]
                               + [(na + g0, True) for g0 in range(0, nb, KGRP)])
                    for g0, isb in gstarts:
                        within = g0 - na if isb else g0
                        k = min(KGRP, (nb - within) if isb else (na - within))
                        ck0 = ch_off + g0
                        eaW_t = eap.tile([128, KGRP * 128], bf16, tag="eaw")
                        nc.sync.dma_start(
                            eaW_t[:, 0:k * 128],
                            d['eaW'].ap()[:, ck0 * 128:(ck0 + k) * 128])
                        ind_t = indp.tile([128, KGRP * 128], bf16, tag="indt")
                        nc.sync.dma_start(
                            ind_t[:, 0:k * 128],
                            d['ind'].ap()[:, ck0 * 128:(ck0 + k) * 128])
                        slot0 = (sb + within) if isb else (sa + within)
                        xg3 = (xgb_flat if isb else xga_flat)[
                            :, slot0 * 128:(slot0 + k) * 128]
                        z_t = wk.tile([128, KGRP * 128], bf16, tag="z")
                        nc.vector.tensor_tensor(z_t[:, 0:k * 128], xg3,
                                                eaW_t[:, 0:k * 128], ALU.add)
                        r_t = wk.tile([128, KGRP * 128], bf16, tag="r")
                        nc.vector.tensor_tensor(r_t[:, 0:k * 128],
                                                z_t[:, 0:k * 128],
                                                c_zb[:, 0:k * 128], ALU.max)
                        wcat = wcatp.tile([128, 2, KGRP * 128], bf16, tag="wcat")
                        nc.scalar.activation(wcat[:, 0, 0:k * 128],
                                             r_t[:, 0:k * 128], AF.Exp,
                                             scale=t_imm)
                        nc.vector.tensor_tensor(wcat[:, 1, 0:k * 128],
                                                wcat[:, 0, 0:k * 128],
                                                r_t[:, 0:k * 128], ALU.mult)
                        for j in range(k):
                            nc.tensor.matmul(
                                acc[:], ind_t[:, j * 128:(j + 1) * 128],
                                wcat[:, :, j * 128:(j + 1) * 128],
                                start=(g0 + j == 0), stop=(g0 + j == nchb - 1))

                    # ---- finalize + node phase ----
                    if 'acc0' in taps and li == 0 and b == 0:
                        tap_t = wk.tile([128, 256], f32, tag="tapacc")
                        nc.vector.tensor_copy(tap_t[:], acc[:])
                        nc.sync.dma_start(taps['acc0'].ap(), tap_t[:])
                    s_t = wk.tile([128, 128], f32, tag="s")
                    nc.vector.tensor_scalar_max(s_t[:], acc[:, 0:128], 1e-20)
                    rec = wk.tile([128, 128], f32, tag="rec")
                    nc.vector.reciprocal_approx_fast(rec[:], s_t[:])
                    o_t = nodep.tile([128, 128], f32, tag="o")
                    nc.vector.tensor_tensor(o_t[:], acc[:, 128:256], rec[:],
                                            ALU.mult)
                    nc.vector.tensor_add(o_t[:], o_t[:],
                                         hb16[:, b * 128:(b + 1) * 128])

                    # node phase
                    tps = npsum.tile([128, 256], f32, tag="nps")
                    nc.tensor.transpose(tps[:, 0:128], o_t[:], c_I[:])
                    oT = nodep.tile([128, 128], f32, tag="oT")
                    nc.scalar.copy(oT[:], tps[:, 0:128])
                    ps1 = npsum.tile([128, 256], f32, tag="nps")
                    nc.tensor.matmul(ps1[:], oT[:],
                                     c_W1[:, li * 256:(li + 1) * 256],
                                     start=True, stop=True)
                    st = nodep.tile([128, 6], f32, tag="st1")
                    nc.vector.bn_stats(st[:], ps1[:])
                    mv = nodep.tile([128, 2], f32, tag="mv1")
                    nc.vector.bn_aggr(mv[:], st[:])
                    rs1 = rsqrt_eps(mv, "1")
                    nmb1 = nodep.tile([128, 1], f32, tag="nm1")
                    nc.vector.tensor_scalar(nmb1[:], mv[:, 0:1], rs1[:, 0:1], -1.0,
                                            ALU.mult, ALU.mult)
                    h1 = nodep.tile([128, 256], f32, tag="h1")
                    nc.scalar.activation(h1[:], ps1[:], AF.Relu,
                                         bias=nmb1[:, 0:1], scale=rs1[:, 0:1])
                    h1T = nodep.tile([128, 256], f32, tag="h1T")
                    for hh in range(2):
                        tps2 = npsum.tile([128, 256], f32, tag="nps")
                        nc.tensor.transpose(tps2[:, 0:128],
                                            h1[:, hh * 128:(hh + 1) * 128], c_I[:])
                        nc.scalar.copy(h1T[:, hh * 128:(hh + 1) * 128],
                                       tps2[:, 0:128])
                    ps2 = npsum.tile([128, 256], f32, tag="nps")
                    for hh in range(2):
                        nc.tensor.matmul(
                            ps2[:, 0:128], h1T[:, hh * 128:(hh + 1) * 128],
                            c_W2[:, li * 256 + hh * 128:li * 256 + (hh + 1) * 128],
                            start=(hh == 0), stop=(hh == 1))
                    xblk = xres[:, b * 128:(b + 1) * 128]
                    if li == 0:
                        nc.scalar.copy(xblk, ps2[:, 0:128])
                    else:
                        nc.vector.tensor_add(xblk, xblk, ps2[:, 0:128])

                    if li + 1 < n_layers:
                        ln_relu(xblk, hb16[:, b * 128:(b + 1) * 128], "t")
                    elif n_layers == L:
                        xf = nodep.tile([128, 128], f32, tag="xf")
                        ln_relu(xblk, xf[:], "f")
                        tps3 = npsum.tile([128, 256], f32, tag="nps")
                        nc.tensor.transpose(tps3[:, 0:128], xf[:], c_I[:])
                        xfT = nodep.tile([128, 128], f32, tag="xfT")
                        nc.scalar.copy(xfT[:], tps3[:, 0:128])
                        ph = npsum.tile([128, 256], f32, tag="nps")
                        nc.tensor.matmul(ph[:, 0:128], xfT[:], c_hW0a[:],
                                         start=True, stop=True)
                        gfb_t = eap.tile([128, 128], f32, tag="gfbt", name="gfb_t")
                        nc.sync.dma_start(gfb_t[:],
                                          d['gfb'].ap()[:, b * 128:(b + 1) * 128])
                        hh1 = nodep.tile([128, 128], f32, tag="hh1")
                        nc.vector.tensor_add(hh1[:], ph[:, 0:128], gfb_t[:])
                        nc.vector.tensor_scalar_max(hh1[:], hh1[:], 0.0)
                        tps4 = npsum.tile([128, 256], f32, tag="nps")
                        nc.tensor.transpose(tps4[:, 0:128], hh1[:], c_I[:])
                        hh1T = nodep.tile([128, 128], f32, tag="hh1T")
                        nc.scalar.copy(hh1T[:], tps4[:, 0:128])
                        po = npsum.tile([128, 256], f32, tag="nps")
                        nc.tensor.matmul(po[:, 0:1], hh1T[:], c_hW1[:],
                                         start=True, stop=True)
                        ocol = nodep.tile([128, 1], f32, tag="ocol")
                        nc.vector.tensor_copy(ocol[:], po[:, 0:1])
                        nc.sync.dma_start(d_out.ap()[:, b:b + 1], ocol[:])

                    if li + 1 < n_layers and b == NA_BLK - 1:
                        # burst-prefetch ahead of the collective: gathers
                        # issued after it on gpsimd stall until it completes
                        ensure_A(li, si + 1 + BURST_A)
                        ensure_B(li, si + 1 + BURST_B)
                        ag_A(tablesA[(li + 1) % 2])

                    sa += na
                    sb += nb
                    ch_off += nchb

            if 'x_l%d' % li in taps:
                tap_t = taps['x_l%d' % li]
                nc.sync.dma_start(tap_t.ap(), xres[:])
            if li + 1 < n_layers:
                # next layer's A-half reads tablesA[(li+1)%2], complete since
                # mid-layer; issue its gathers before ag_B so the A-half edge
                # compute rides out the collective.
                ensure_A(li + 1, BURST_A)
                ag_B(tablesB[(li + 1) % 2])

    nc.compile()
    return nc, taps


# --------------------------------------------------------------------------
# entry point
# --------------------------------------------------------------------------

def _in_maps(blocks, per_core, W):
    shared = dict(
        node_W=W['node_W'], I128=W['I128'], W1s=W['W1s'], W2s=W['W2s'],
        head_W0a=W['head_W0a'], head_W1=W['head_W1'])
    return [dict(per_core[c], **shared) for c in range(NC)]


def kernel(**inputs):
    from concourse import bass_utils
    blocks, per_core, W = _prep(inputs)
    nc, _ = _build(blocks, W)
    res = bass_utils.run_bass_kernel_spmd(
        nc, _in_maps(blocks, per_core, W), core_ids=list(range(NC)),
        trace=False)
    out = np.empty((N, 1), np.float32)
    for c in range(NC):
        oc = res.results[c]['out']
        out[c * NPC:(c + 1) * NPC, 0] = oc.T.reshape(-1)[:NPC] + W['head_b1']
    return out
